# revision 23
# baseline (speedup 1.0000x reference)
"""CharRNN Trainium2 kernel.

Math (per batch row b):
    x_proj = emb_table[x] @ W_e            # == (emb_table @ W_e)[x]  (gather commutes)
    h_t    = tanh(x_proj[t] + h_{t-1} @ W_h)
    logits = outs @ W_o

Strategy: data-parallel over batch across 8 cores (32 rows each). On each
core the hidden state is kept TRANSPOSED (H on partitions, batch on free
dim, 8 chunks of [128, 32] in an SBUF ring) so the recurrence matmul needs
no per-step transpose of its stationary operand:

    z[b, n] = sum_k ht[k].T @ W_h[k, n]    lhsT = ht chunk [128, 32]
                                           rhs  = W_h chunk [128, 512] fp32r

x_proj is injected into the PSUM accumulation by an identity matmul
(start=True) reading just-in-time x_proj tiles. Those tiles are produced
on-device from embW = emb_table @ W_e via one-hot matmuls (a single
is_equal against an iota table builds the one-hot for 128 tokens = 4
steps), avoiding indirect-DMA gathers which are extremely slow in this
environment. tanh output [32, 1024] returns to ring layout with 8 PE
transposes per step. The output projection runs every 16 steps as a
batched matmul over the ring (N=512), transposed back to batch-major on
the PE and DMAed straight from PSUM.
"""

from contextlib import ExitStack

import numpy as np
import concourse.bass as bass
import concourse.tile as tile
from concourse import bacc, mybir
from concourse.bass_utils import run_bass_kernel_spmd
from concourse.vector_clock import ScopedClock
from concourse.masks import make_identity

P = 128
B, L, V, E, H = 256, 512, 256, 256, 1024
NCORES = 8
BL = B // NCORES          # 32 batch rows per core
KC = H // P               # 8 contraction chunks
F32 = mybir.dt.float32
F32R = mybir.dt.float32r
I32 = mybir.dt.int32
TANH = mybir.ActivationFunctionType.Tanh


class _TC(tile.TileContext):
    """Walrus in this build lowers InstDrain with at most ONE sync wait
    (NEURON_ISA_TPB_CTRL_NO_STRUCT). Split the exit drain's global-clock
    waits across a chain of single-wait drains."""

    def _drain_and_barrier(self, tick_clock, wait_clock):
        nc = self.nc
        drain_inst = nc.sync.drain()
        wait_clock.add_sem_waits(
            drain_inst.ins, ScopedClock({None: tick_clock.global_clock})
        )
        si = drain_inst.ins.sync_info
        if si is not None and len(si.on_wait) > 1:
            waits = list(si.on_wait)
            upd = list(si.on_update)
            drain_inst.ins.sync_info = mybir.SyncInfo(on_wait=waits[:1], on_update=upd)
            for i in range(1, len(waits)):
                d2 = nc.sync.drain()
                d2.ins.sync_info = mybir.SyncInfo(on_wait=[waits[i]], on_update=[])
        nc.all_engine_barrier()
        popped = nc._tile_sem_poison_stack.pop()
        assert popped is self._sem_poison
        nc.clear_and_free_semaphores(list(self.sems.allocated().values()))
        nc.all_engine_barrier()


def build(L_steps=L, blk=16, repeat=1, abl=0):
    """Build the per-core Bass program (SPMD: all cores run this).
    repeat>1 reruns the main loop (timing experiments only)."""
    assert L_steps % blk == 0 and L_steps % 4 == 0 and blk % 4 == 0
    slots = 2 * blk
    n_xtile = L_steps // 4  # one x-token tile = 128 tokens = 4 steps

    nc = bacc.Bacc("TRN2", target_bir_lowering=False, debug=False, num_devices=NCORES)
    x_d = nc.dram_tensor("x", [BL, L_steps], I32, kind="ExternalInput").ap()
    h0_d = nc.dram_tensor("hidden", [BL, H], F32, kind="ExternalInput").ap()
    et_d = nc.dram_tensor("emb_table", [V, E], F32, kind="ExternalInput").ap()
    we_d = nc.dram_tensor("W_e", [E, H], F32, kind="ExternalInput").ap()
    wh_d = nc.dram_tensor("W_h", [H, H], F32, kind="ExternalInput").ap()
    wo_d = nc.dram_tensor("W_o", [H, V], F32, kind="ExternalInput").ap()
    # kernel-native layout; host unshard transposes to [BL, L, V]
    lg_d = nc.dram_tensor("logits_raw", [2, P, L_steps, BL], F32, kind="ExternalOutput").ap()
    hf_d = nc.dram_tensor("hfinal", [BL, H], F32, kind="ExternalOutput").ap()

    with _TC(nc) as tc, ExitStack() as ctx:
        pers = ctx.enter_context(tc.tile_pool(name="pers", bufs=1))

        # --- persistent SBUF tensors (matmul operands are F32R: walrus
        # requires fp32r-consumed data to come from a rounding op, so DMA
        # into f32 staging then engine-copy with dtype conversion)
        wh_sb = pers.tile([P, KC * H], F32R)  # W_h chunk k at cols [H*k, H*k+H)
        wo_sb = pers.tile([P, KC * V], F32R)  # W_o chunk k at cols [V*k, ...)
        embWr = pers.tile([P, 2 * H], F32R)  # embW v-chunk c at cols [H*c, ...)
        etab_sb = pers.tile([P, 2 * E], F32)  # emb_table chunk c at cols [E*c, ...)
        nc.sync.dma_start(
            etab_sb[:].rearrange("p (c e) -> p c e", c=2),
            et_d.rearrange("(c p) e -> p c e", p=P),
        )
        ident = pers.tile([P, P], F32)
        make_identity(nc, ident[:])
        # token ids, interleaved so tile i partition (32*ti + b) = x[b, 4i+ti]
        x4 = pers.tile([P, n_xtile], I32)
        for ti in range(4):
            nc.sync.dma_start(
                x4[ti * BL : (ti + 1) * BL, :],
                x_d[:].rearrange("b (i ti) -> b i ti", ti=4)[:, :, ti],
            )
        iota_b = pers.tile([P, V], I32)  # every partition: 0..255
        nc.gpsimd.iota(iota_b[:], pattern=[[1, V]], base=0, channel_multiplier=0)
        # ring of transposed hidden states: chunk k, slot s at
        # cols [k*slots*BL + s*BL, ... + BL)
        ring = pers.tile([P, KC * slots * BL], F32R)

        # --- startup: embWr = emb_table @ W_e (stays in SBUF)
        with tc.tile_pool(name="init_ps", bufs=2, space="PSUM") as ips, tc.tile_pool(
            name="init_sb", bufs=1
        ) as isb:
            stage = isb.tile([P, KC * H], F32, tag="stage")
            nc.sync.dma_start(
                stage[:].rearrange("p (k n) -> p k n", k=KC),
                wh_d.rearrange("(k p) n -> p k n", p=P),
            )
            nc.vector.tensor_copy(wh_sb[:], stage[:])
            stage = isb.tile([P, KC * H], F32, tag="stage", name="stage2")
            nc.sync.dma_start(
                stage[:, : KC * V].rearrange("p (k v) -> p k v", k=KC),
                wo_d.rearrange("(k p) v -> p k v", p=P),
            )
            nc.scalar.copy(wo_sb[:], stage[:, : KC * V])
            stage = isb.tile([P, KC * H], F32, tag="stage", name="stage3")
            nc.sync.dma_start(
                stage[:, : 2 * H].rearrange("p (e h) -> p e h", e=2),
                we_d.rearrange("(e p) h -> p e h", p=P),
            )
            we_r = isb.tile([P, 2 * H], F32R, tag="wer")
            nc.scalar.copy(we_r[:], stage[:, : 2 * H])
            etT = isb.tile([P, 2 * V], F32R, tag="etT")  # e-chunk ec at [V*ec, ...)
            for c in range(2):  # v chunk of emb_table rows
                for ec in range(2):  # e chunk
                    ptr0 = ips.tile([P, P], F32, tag="ptr0")
                    nc.tensor.transpose(
                        ptr0[:],
                        etab_sb[:, c * E + ec * P : c * E + (ec + 1) * P],
                        ident[:],
                    )
                    nc.vector.tensor_copy(
                        etT[:, ec * V + c * P : ec * V + (c + 1) * P], ptr0[:]
                    )
            for vc in range(2):  # embW row chunk
                pe_ps = ips.tile([P, H], F32, tag="pe_ps")
                for nh in range(2):  # embW col half
                    for ec in range(2):
                        nc.tensor.matmul(
                            pe_ps[:, nh * 512 : (nh + 1) * 512],
                            lhsT=etT[:, ec * V + vc * P : ec * V + (vc + 1) * P],
                            rhs=we_r[:, ec * H + nh * 512 : ec * H + (nh + 1) * 512],
                            start=(ec == 0),
                            stop=(ec == 1),
                        )
                nc.vector.tensor_copy(embWr[:, vc * H : (vc + 1) * H], pe_ps[:])

        # --- main pools (PSUM budget: 3 + 2 + 2 + 1 = 8 banks)
        gat = ctx.enter_context(tc.tile_pool(name="gat", bufs=2))
        work = ctx.enter_context(tc.tile_pool(name="work", bufs=4))
        psz = ctx.enter_context(tc.tile_pool(name="psz", bufs=2, space="PSUM"))
        pst = ctx.enter_context(tc.tile_pool(name="pst", bufs=1, space="PSUM"))
        pxp = ctx.enter_context(tc.tile_pool(name="pxp", bufs=1, space="PSUM"))
        psp = ctx.enter_context(tc.tile_pool(name="psp", bufs=1, space="PSUM"))

        def make_xp(i):
            """x_proj for token tile i (steps 4i..4i+3): [128 (ti,b), 1024] F32R."""
            oh = work.tile([P, V], F32, tag="oh")
            nc.vector.tensor_tensor(
                out=oh[:],
                in0=x4[:, i : i + 1].to_broadcast([P, V]),
                in1=iota_b[:],
                op=mybir.AluOpType.is_equal,
            )
            ohT = work.tile([P, 2 * P], F32R, tag="ohT")
            for c in range(2):
                pt = pst.tile([P, P], F32, tag="ptr", name="ohT_ps")
                nc.tensor.transpose(pt[:], oh[:, c * P : (c + 1) * P], ident[:])
                nc.vector.tensor_copy(ohT[:, c * P : (c + 1) * P], pt[:])
            ppx = pxp.tile([P, H], F32, tag="ppx")
            for nh in range(2):
                for c in range(2):
                    nc.tensor.matmul(
                        ppx[:, nh * 512 : (nh + 1) * 512],
                        lhsT=ohT[:, c * P : (c + 1) * P],
                        rhs=embWr[:, c * H + nh * 512 : c * H + (nh + 1) * 512],
                        start=(c == 0),
                        stop=(c == 1),
                    )
            xp = gat.tile([P, H], F32, tag="xp")
            nc.vector.tensor_copy(xp[:], ppx[:])
            return xp

        def transpose_to_ring(h_ap, slot):
            """h [32, 1024] (batch on partitions) -> ring chunks [128, 32] at slot.
            DVE 32x32 block-transpose + 4 partition-shifted strided copies
            (avoids 8 PE transposes; matmuls are the expensive instruction
            on this backend)."""
            htB = work.tile([BL, H], F32, tag="htB")
            nc.vector.transpose(htB[:], h_ap[:])
            rv = ring[:].rearrange("p (k sb) -> p k sb", k=KC)
            sv = htB[:].rearrange("c (k j r) -> c k j r", k=KC, j=4)
            for jl in range(4):
                nc.vector.tensor_copy(
                    rv[jl * BL : (jl + 1) * BL, :, slot * BL : slot * BL + BL],
                    sv[:, :, jl, :],
                )

        # initial hidden -> ring slot (slots-1)
        h0_sb = work.tile([BL, H], F32, tag="h_sb")
        nc.sync.dma_start(h0_sb[:], h0_d[:])
        transpose_to_ring(h0_sb, slots - 1)

        xp4 = make_xp(0)
        prev_slot = slots - 1
        h_sb = None
        for t_rep in range(L_steps * repeat):
            t = t_rep % L_steps
            slot = t % slots
            ti = t % 4
            if ti == 0 and t > 0 and not (abl & 16):
                xp4 = make_xp(t // 4)

            # z = ht_{t-1}.T @ W_h  (x_proj added on DVE afterwards)
            ps = psz.tile([BL, H], F32, tag="psz")
            base = prev_slot * BL
            for nh in range(2):
                for k in range(0 if not (abl & 1) else KC, KC):
                    nc.tensor.matmul(
                        ps[:, nh * 512 : (nh + 1) * 512],
                        lhsT=ring[:, k * slots * BL + base : k * slots * BL + base + BL],
                        rhs=wh_sb[:, k * H + nh * 512 : k * H + nh * 512 + 512],
                        start=(k == 0),
                        stop=(k == KC - 1),
                    )
            h_sb = work.tile([BL, H], F32, tag="h_sb")
            if not (abl & 2):
                zt = work.tile([BL, H], F32, tag="zt")
                nc.vector.tensor_add(
                    zt[:], ps[:], xp4[ti * BL : (ti + 1) * BL, :]
                )
                nc.scalar.activation(h_sb[:], zt[:], TANH)
            if not (abl & 4):
                transpose_to_ring(h_sb, slot)
            prev_slot = slot

            # output projection over the last `blk` steps
            if t % blk == blk - 1 and not (abl & 8):
                bi = t // blk
                par = (bi % 2) * blk * BL
                for mv in range(2):
                    pp = psp.tile([P, blk * BL], F32, tag="pp")
                    for k in range(KC):
                        nc.tensor.matmul(
                            pp[:],
                            lhsT=wo_sb[:, k * V + mv * P : k * V + (mv + 1) * P],
                            rhs=ring[:, k * slots * BL + par : k * slots * BL + par + blk * BL],
                            start=(k == 0),
                            stop=(k == KC - 1),
                        )
                    lg_sb = work.tile([P, blk * BL], F32, tag="lg", name=f"lg{mv}")
                    nc.vector.tensor_copy(lg_sb[:], pp[:])
                    nc.sync.dma_start(
                        lg_d[mv, :, bi * blk : (bi + 1) * blk, :],
                        lg_sb[:].rearrange("p (s b) -> p s b", s=blk),
                    )

        if h_sb is not None:
            nc.sync.dma_start(hf_d[:], h_sb[:])
        else:
            nc.sync.dma_start(hf_d[:], h0_sb[:])
    nc.compile()
    return nc


_CACHE = {}


def _get_nc():
    if "nc" not in _CACHE:
        _CACHE["nc"] = build()
    return _CACHE["nc"]


def kernel(x, hidden, emb_table, W_e, W_h, W_o):
    x = np.ascontiguousarray(np.asarray(x).astype(np.int32))
    hidden = np.ascontiguousarray(np.asarray(hidden, dtype=np.float32))
    emb_table = np.ascontiguousarray(np.asarray(emb_table, dtype=np.float32))
    W_e = np.ascontiguousarray(np.asarray(W_e, dtype=np.float32))
    W_h = np.ascontiguousarray(np.asarray(W_h, dtype=np.float32))
    W_o = np.ascontiguousarray(np.asarray(W_o, dtype=np.float32))

    nc = _get_nc()
    in_maps = []
    for c in range(NCORES):
        sl = slice(c * BL, (c + 1) * BL)
        in_maps.append(
            {
                "x": x[sl],
                "hidden": hidden[sl],
                "emb_table": emb_table,
                "W_e": W_e,
                "W_h": W_h,
                "W_o": W_o,
            }
        )
    res = run_bass_kernel_spmd(nc, in_maps, core_ids=list(range(NCORES)), trace=False)
    # logits_raw[mv, p, t, b] = logits[b, t, mv*128 + p]
    logits = np.concatenate(
        [
            res.results[c]["logits_raw"].transpose(3, 2, 0, 1).reshape(BL, L, V)
            for c in range(NCORES)
        ],
        axis=0,
    )
    hfinal = np.concatenate([res.results[c]["hfinal"] for c in range(NCORES)], axis=0)
    return logits, hfinal


# revision 24
# speedup vs baseline: 2.5061x; 2.5061x over previous
"""CharRNN Trainium2 kernel.

Math (per batch row b):
    x_proj = emb_table[x] @ W_e            # == (emb_table @ W_e)[x]  (gather commutes)
    h_t    = tanh(x_proj[t] + h_{t-1} @ W_h)
    logits = outs @ W_o

Strategy: data-parallel over batch across 8 cores (32 rows each). On each
core the hidden state is kept TRANSPOSED (H on partitions, batch on free
dim, 8 chunks of [128, 32] in an SBUF ring) so the recurrence matmul needs
no per-step transpose of its stationary operand:

    z[b, n] = sum_k ht[k].T @ W_h[k, n]    lhsT = ht chunk [128, 32]
                                           rhs  = W_h chunk [128, 512] fp32r

x_proj is injected into the PSUM accumulation by an identity matmul
(start=True) reading just-in-time x_proj tiles. Those tiles are produced
on-device from embW = emb_table @ W_e via one-hot matmuls (a single
is_equal against an iota table builds the one-hot for 128 tokens = 4
steps), avoiding indirect-DMA gathers which are extremely slow in this
environment. tanh output [32, 1024] returns to ring layout with 8 PE
transposes per step. The output projection runs every 16 steps as a
batched matmul over the ring (N=512), transposed back to batch-major on
the PE and DMAed straight from PSUM.
"""

from contextlib import ExitStack

import numpy as np
import concourse.bass as bass
import concourse.tile as tile
from concourse import bacc, mybir
from concourse.bass_utils import run_bass_kernel_spmd
from concourse.vector_clock import ScopedClock
from concourse.masks import make_identity

P = 128
B, L, V, E, H = 256, 512, 256, 256, 1024
NCORES = 8
BL = B // NCORES          # 32 batch rows per core
KC = H // P               # 8 contraction chunks
F32 = mybir.dt.float32
F32R = mybir.dt.float32r
I32 = mybir.dt.int32
TANH = mybir.ActivationFunctionType.Tanh


class _TC(tile.TileContext):
    """Walrus in this build lowers InstDrain with at most ONE sync wait
    (NEURON_ISA_TPB_CTRL_NO_STRUCT). Split the exit drain's global-clock
    waits across a chain of single-wait drains."""

    def _drain_and_barrier(self, tick_clock, wait_clock):
        nc = self.nc
        drain_inst = nc.sync.drain()
        wait_clock.add_sem_waits(
            drain_inst.ins, ScopedClock({None: tick_clock.global_clock})
        )
        si = drain_inst.ins.sync_info
        if si is not None and len(si.on_wait) > 1:
            waits = list(si.on_wait)
            upd = list(si.on_update)
            drain_inst.ins.sync_info = mybir.SyncInfo(on_wait=waits[:1], on_update=upd)
            for i in range(1, len(waits)):
                d2 = nc.sync.drain()
                d2.ins.sync_info = mybir.SyncInfo(on_wait=[waits[i]], on_update=[])
        nc.all_engine_barrier()
        popped = nc._tile_sem_poison_stack.pop()
        assert popped is self._sem_poison
        nc.clear_and_free_semaphores(list(self.sems.allocated().values()))
        nc.all_engine_barrier()


def build(L_steps=L, blk=16, repeat=1, abl=0):
    """Build the per-core Bass program (SPMD: all cores run this).
    repeat>1 reruns the main loop (timing experiments only)."""
    assert L_steps % blk == 0 and L_steps % 4 == 0 and blk % 4 == 0
    slots = 2 * blk
    n_xtile = L_steps // 4  # one x-token tile = 128 tokens = 4 steps

    nc = bacc.Bacc("TRN2", target_bir_lowering=False, debug=False, num_devices=NCORES)
    x_d = nc.dram_tensor("x", [BL, L_steps], I32, kind="ExternalInput").ap()
    h0_d = nc.dram_tensor("hidden", [BL, H], F32, kind="ExternalInput").ap()
    et_d = nc.dram_tensor("emb_table", [V, E], F32, kind="ExternalInput").ap()
    we_d = nc.dram_tensor("W_e", [E, H], F32, kind="ExternalInput").ap()
    wh_d = nc.dram_tensor("W_h", [H, H], F32, kind="ExternalInput").ap()
    wo_d = nc.dram_tensor("W_o", [H, V], F32, kind="ExternalInput").ap()
    # kernel-native layout; host unshard transposes to [BL, L, V]
    lg_d = nc.dram_tensor("logits_raw", [2, P, L_steps, BL], F32, kind="ExternalOutput").ap()
    hf_d = nc.dram_tensor("hfinal", [BL, H], F32, kind="ExternalOutput").ap()

    with _TC(nc) as tc, ExitStack() as ctx:
        pers = ctx.enter_context(tc.tile_pool(name="pers", bufs=1))

        # --- persistent SBUF tensors (matmul operands are F32R: walrus
        # requires fp32r-consumed data to come from a rounding op, so DMA
        # into f32 staging then engine-copy with dtype conversion)
        wh_sb = pers.tile([P, KC * H], F32R)  # W_h chunk k at cols [H*k, H*k+H)
        wo_sb = pers.tile([P, KC * V], F32R)  # W_o chunk k at cols [V*k, ...)
        embWr = pers.tile([P, 2 * H], F32R)  # embW v-chunk c at cols [H*c, ...)
        etab_sb = pers.tile([P, 2 * E], F32)  # emb_table chunk c at cols [E*c, ...)
        nc.sync.dma_start(
            etab_sb[:].rearrange("p (c e) -> p c e", c=2),
            et_d.rearrange("(c p) e -> p c e", p=P),
        )
        ident = pers.tile([P, P], F32)
        make_identity(nc, ident[:])
        # token ids, interleaved so tile i partition (32*ti + b) = x[b, 4i+ti]
        x4 = pers.tile([P, n_xtile], I32)
        for ti in range(4):
            nc.sync.dma_start(
                x4[ti * BL : (ti + 1) * BL, :],
                x_d[:].rearrange("b (i ti) -> b i ti", ti=4)[:, :, ti],
            )
        iota_b = pers.tile([P, V], I32)  # every partition: 0..255
        nc.gpsimd.iota(iota_b[:], pattern=[[1, V]], base=0, channel_multiplier=0)
        # ring of transposed hidden states: chunk k, slot s at
        # cols [k*slots*BL + s*BL, ... + BL)
        ring = pers.tile([P, KC * slots * BL], F32R)

        # --- startup: embWr = emb_table @ W_e (stays in SBUF)
        with tc.tile_pool(name="init_ps", bufs=2, space="PSUM") as ips, tc.tile_pool(
            name="init_sb", bufs=1
        ) as isb:
            stage = isb.tile([P, KC * H], F32, tag="stage")
            nc.sync.dma_start(
                stage[:].rearrange("p (k n) -> p k n", k=KC),
                wh_d.rearrange("(k p) n -> p k n", p=P),
            )
            nc.vector.tensor_copy(wh_sb[:], stage[:])
            stage = isb.tile([P, KC * H], F32, tag="stage", name="stage2")
            nc.sync.dma_start(
                stage[:, : KC * V].rearrange("p (k v) -> p k v", k=KC),
                wo_d.rearrange("(k p) v -> p k v", p=P),
            )
            nc.scalar.copy(wo_sb[:], stage[:, : KC * V])
            stage = isb.tile([P, KC * H], F32, tag="stage", name="stage3")
            nc.sync.dma_start(
                stage[:, : 2 * H].rearrange("p (e h) -> p e h", e=2),
                we_d.rearrange("(e p) h -> p e h", p=P),
            )
            we_r = isb.tile([P, 2 * H], F32R, tag="wer")
            nc.scalar.copy(we_r[:], stage[:, : 2 * H])
            etT = isb.tile([P, 2 * V], F32R, tag="etT")  # e-chunk ec at [V*ec, ...)
            for c in range(2):  # v chunk of emb_table rows
                for ec in range(2):  # e chunk
                    ptr0 = ips.tile([P, P], F32, tag="ptr0")
                    nc.tensor.transpose(
                        ptr0[:],
                        etab_sb[:, c * E + ec * P : c * E + (ec + 1) * P],
                        ident[:],
                    )
                    nc.vector.tensor_copy(
                        etT[:, ec * V + c * P : ec * V + (c + 1) * P], ptr0[:]
                    )
            for vc in range(2):  # embW row chunk
                pe_ps = ips.tile([P, H], F32, tag="pe_ps")
                for nh in range(2):  # embW col half
                    for ec in range(2):
                        nc.tensor.matmul(
                            pe_ps[:, nh * 512 : (nh + 1) * 512],
                            lhsT=etT[:, ec * V + vc * P : ec * V + (vc + 1) * P],
                            rhs=we_r[:, ec * H + nh * 512 : ec * H + (nh + 1) * 512],
                            start=(ec == 0),
                            stop=(ec == 1),
                        )
                nc.vector.tensor_copy(embWr[:, vc * H : (vc + 1) * H], pe_ps[:])

        # --- main pools (PSUM budget: 3 + 2 + 2 + 1 = 8 banks)
        gat = ctx.enter_context(tc.tile_pool(name="gat", bufs=2))
        work = ctx.enter_context(tc.tile_pool(name="work", bufs=4))
        psz = ctx.enter_context(tc.tile_pool(name="psz", bufs=2, space="PSUM"))
        pst = ctx.enter_context(tc.tile_pool(name="pst", bufs=1, space="PSUM"))
        pxp = ctx.enter_context(tc.tile_pool(name="pxp", bufs=1, space="PSUM"))
        psp = ctx.enter_context(tc.tile_pool(name="psp", bufs=1, space="PSUM"))

        def make_xp(i):
            """x_proj for token tile i (steps 4i..4i+3): [128 (ti,b), 1024] F32R."""
            oh = work.tile([P, V], F32, tag="oh")
            nc.vector.tensor_tensor(
                out=oh[:],
                in0=x4[:, i : i + 1].to_broadcast([P, V]),
                in1=iota_b[:],
                op=mybir.AluOpType.is_equal,
            )
            ohT = work.tile([P, 2 * P], F32R, tag="ohT")
            for c in range(2):
                pt = pst.tile([P, P], F32, tag="ptr", name="ohT_ps")
                nc.tensor.transpose(pt[:], oh[:, c * P : (c + 1) * P], ident[:])
                nc.vector.tensor_copy(ohT[:, c * P : (c + 1) * P], pt[:])
            ppx = pxp.tile([P, H], F32, tag="ppx")
            for nh in range(2):
                for c in range(2):
                    nc.tensor.matmul(
                        ppx[:, nh * 512 : (nh + 1) * 512],
                        lhsT=ohT[:, c * P : (c + 1) * P],
                        rhs=embWr[:, c * H + nh * 512 : c * H + (nh + 1) * 512],
                        start=(c == 0),
                        stop=(c == 1),
                    )
            xp = gat.tile([P, H], F32, tag="xp")
            nc.vector.tensor_copy(xp[:], ppx[:])
            return xp

        def transpose_to_ring(h_ap, slot):
            """h [32, 1024] (batch on partitions) -> ring chunks [128, 32] at slot.
            DVE 32x32 block-transpose + 4 partition-shifted strided copies
            (avoids 8 PE transposes; matmuls are the expensive instruction
            on this backend)."""
            htB = work.tile([BL, H], F32, tag="htB")
            nc.vector.transpose(htB[:], h_ap[:])
            rv = ring[:].rearrange("p (k sb) -> p k sb", k=KC)
            sv = htB[:].rearrange("c (k j r) -> c k j r", k=KC, j=4)
            for jl in range(4):
                nc.vector.tensor_copy(
                    rv[jl * BL : (jl + 1) * BL, :, slot * BL : slot * BL + BL],
                    sv[:, :, jl, :],
                )

        # initial hidden -> ring slot (slots-1)
        h0_sb = work.tile([BL, H], F32, tag="h_sb")
        nc.sync.dma_start(h0_sb[:], h0_d[:])
        transpose_to_ring(h0_sb, slots - 1)

        xp4 = make_xp(0)
        prev_slot = slots - 1
        h_sb = None
        for t_rep in range(L_steps * repeat):
            t = t_rep % L_steps
            slot = t % slots
            ti = t % 4
            if ti == 0 and t > 0 and not (abl & 16):
                xp4 = make_xp(t // 4)

            # z = ht_{t-1}.T @ W_h  (x_proj added on DVE afterwards)
            ps = psz.tile([BL, H], F32, tag="psz")
            base = prev_slot * BL
            for nh in range(2):
                ks = list(range(KC)) if not (abl & 1) else [0]
                for k in ks:
                    nc.tensor.matmul(
                        ps[:, nh * 512 : (nh + 1) * 512],
                        lhsT=ring[:, k * slots * BL + base : k * slots * BL + base + BL],
                        rhs=wh_sb[:, k * H + nh * 512 : k * H + nh * 512 + 512],
                        start=(k == ks[0]),
                        stop=(k == ks[-1]),
                    )
            h_sb = work.tile([BL, H], F32, tag="h_sb")
            if not (abl & 2):
                zt = work.tile([BL, H], F32, tag="zt")
                nc.vector.tensor_add(
                    zt[:], ps[:], xp4[ti * BL : (ti + 1) * BL, :]
                )
                nc.scalar.activation(h_sb[:], zt[:], TANH)
            if not (abl & 4):
                transpose_to_ring(h_sb, slot)
            prev_slot = slot

            # output projection over the last `blk` steps
            if t % blk == blk - 1 and not (abl & 8):
                bi = t // blk
                par = (bi % 2) * blk * BL
                for mv in range(2):
                    pp = psp.tile([P, blk * BL], F32, tag="pp")
                    for k in range(KC):
                        nc.tensor.matmul(
                            pp[:],
                            lhsT=wo_sb[:, k * V + mv * P : k * V + (mv + 1) * P],
                            rhs=ring[:, k * slots * BL + par : k * slots * BL + par + blk * BL],
                            start=(k == 0),
                            stop=(k == KC - 1),
                        )
                    lg_sb = work.tile([P, blk * BL], F32, tag="lg", name=f"lg{mv}")
                    nc.vector.tensor_copy(lg_sb[:], pp[:])
                    nc.sync.dma_start(
                        lg_d[mv, :, bi * blk : (bi + 1) * blk, :],
                        lg_sb[:].rearrange("p (s b) -> p s b", s=blk),
                    )

        if h_sb is not None:
            nc.sync.dma_start(hf_d[:], h_sb[:])
        else:
            nc.sync.dma_start(hf_d[:], h0_sb[:])
    nc.compile()
    return nc


_CACHE = {}


def _get_nc():
    if "nc" not in _CACHE:
        _CACHE["nc"] = build()
    return _CACHE["nc"]


def kernel(x, hidden, emb_table, W_e, W_h, W_o):
    x = np.ascontiguousarray(np.asarray(x).astype(np.int32))
    hidden = np.ascontiguousarray(np.asarray(hidden, dtype=np.float32))
    emb_table = np.ascontiguousarray(np.asarray(emb_table, dtype=np.float32))
    W_e = np.ascontiguousarray(np.asarray(W_e, dtype=np.float32))
    W_h = np.ascontiguousarray(np.asarray(W_h, dtype=np.float32))
    W_o = np.ascontiguousarray(np.asarray(W_o, dtype=np.float32))

    nc = _get_nc()
    in_maps = []
    for c in range(NCORES):
        sl = slice(c * BL, (c + 1) * BL)
        in_maps.append(
            {
                "x": x[sl],
                "hidden": hidden[sl],
                "emb_table": emb_table,
                "W_e": W_e,
                "W_h": W_h,
                "W_o": W_o,
            }
        )
    res = run_bass_kernel_spmd(nc, in_maps, core_ids=list(range(NCORES)), trace=False)
    # logits_raw[mv, p, t, b] = logits[b, t, mv*128 + p]
    logits = np.concatenate(
        [
            res.results[c]["logits_raw"].transpose(3, 2, 0, 1).reshape(BL, L, V)
            for c in range(NCORES)
        ],
        axis=0,
    )
    hfinal = np.concatenate([res.results[c]["hfinal"] for c in range(NCORES)], axis=0)
    return logits, hfinal
